# revision 17
# baseline (speedup 1.0000x reference)
"""Trainium2 Bass kernel for 2-layer LSTM (H=32, in=1) + MLP head.

Problem: x [4096, 512, 1] f32 -> y [4096, 1] f32.
Strategy: pure data parallel over 8 cores (512 batch each).

Per-core layout ("orientation B", block-diagonal):
  batch b = g*128 + j  (g in 0..3 groups, j in 0..127)
  partition dim = j always; free dim = (layer, group, gate/hidden)

Per tick n (0..512): layer-0 step n and layer-1 step n-1.
  Gates via matmuls with stationary hT (transposed hidden state) and
  streamed block-diagonal weights [128, 512]; x/bias terms via small-K
  matmuls accumulating into the same PSUM banks.
  Gate order (after host-side reorder): [i, f, o, g] per 128-block; the
  g-gate's weights/biases are pre-scaled by 2 so that one big Sigmoid
  covers all gates (tanh(z) = 2*sigmoid(2z) - 1, fixed up on DVE).
  h -> hT via PE transpose (bf16) + DVE copy back to SBUF.
"""

import os
import numpy as np
import ml_dtypes
from contextlib import ExitStack

import concourse.bass as bass
import concourse.tile as tile
import concourse.bacc as bacc
from concourse import mybir
from concourse import bass_utils

F32 = mybir.dt.float32
BF16 = mybir.dt.bfloat16
AF = mybir.ActivationFunctionType
OP = mybir.AluOpType

H = 32
NCORES = 8
B_FULL = 4096
S_FULL = 512
BC = 512          # batch per core
NG = 4            # groups of 128 within the core batch

# torch gate order i,f,g,o -> ours [i,f,o,g]
GATE_PERM = np.r_[0:32, 32:64, 96:128, 64:96]
# scale: g-gate block (now at 96:128) gets x2 for the sigmoid-only trick
GATE_SCALE = np.ones(128, np.float32)
GATE_SCALE[96:128] = 2.0


def _bf(x):
    return np.asarray(x, dtype=ml_dtypes.bfloat16)


def prep_shared_weights(w_ih0, w_hh0, b_ih0, b_hh0, w_ih1, w_hh1, b_ih1, b_hh1,
                        fc1_w, fc1_b, fc2_w, fc2_b):
    """Host-side preparation of the shared (replicated) weight tensors."""
    def reord(w):  # [128, k] -> perm + scale rows
        return (w[GATE_PERM] * GATE_SCALE[:, None]).astype(np.float32)

    w_hh0p = reord(w_hh0)            # [128, 32]
    w_ih0p = reord(w_ih0)            # [128, 1]
    b0p = ((b_ih0 + b_hh0)[GATE_PERM] * GATE_SCALE).astype(np.float32)  # [128]
    w_hh1p = reord(w_hh1)
    w_ih1p = reord(w_ih1)
    b1p = ((b_ih1 + b_hh1)[GATE_PERM] * GATE_SCALE).astype(np.float32)

    def blockdiag(wp):  # wp [128 gates, K] -> [NG*K, NG*128]
        k = wp.shape[1]
        out = np.zeros((NG * k, NG * 128), np.float32)
        for g in range(NG):
            out[g * k:(g + 1) * k, g * 128:(g + 1) * 128] = wp.T
        return out

    wbd_hh0 = blockdiag(w_hh0p)      # [128, 512]
    wbd_hh1 = blockdiag(w_hh1p)      # [128, 512]
    wbd_ih1 = blockdiag(w_ih1p)      # [128, 512]
    # const rhs for layer0: rows 0..3 = x block-diag w_ih0 row, row 4 = bias0
    rc0 = np.zeros((5, NG * 128), np.float32)
    for g in range(NG):
        rc0[g, g * 128:(g + 1) * 128] = w_ih0p[:, 0]
    rc0[4] = np.tile(b0p, NG)
    rc1 = np.tile(b1p, NG)[None, :]  # [1, 512]

    return {
        "wbd_hh0": _bf(wbd_hh0),
        "wbd_hh1": _bf(wbd_hh1),
        "wbd_ih1": _bf(wbd_ih1),
        "rc0": _bf(rc0),
        "rc1": _bf(rc1),
        "id128": _bf(np.eye(128, dtype=np.float32)),
        "fc1T": _bf(fc1_w.T.copy()),               # [32, 16]
        "fc1b": fc1_b.reshape(16, 1).astype(np.float32),
        "fc2T": _bf(fc2_w.T.copy()),               # [16, 1]
        "fc2b": fc2_b.reshape(1, 1).astype(np.float32),
    }


def prep_core_x(x_core, s_len):
    """x_core [512, s_len] f32 -> xs [5, s_len*128] bf16 (rows 0..3 x per
    group time-major, row 4 ones)."""
    xs = np.ones((5, s_len * 128), np.float32)
    xr = x_core.reshape(NG, 128, s_len)             # [g, j, t]
    xs[:4] = xr.transpose(0, 2, 1).reshape(NG, s_len * 128)  # [g, t*128+j]
    return _bf(xs)


def build_program(s_len, num_devices=NCORES):
    nc = bacc.Bacc("TRN2", target_bir_lowering=False, debug=False,
                   enable_asserts=False, num_devices=num_devices)
    d = {}
    def din(name, shape, dt):
        d[name] = nc.dram_tensor(name, shape, dt, kind="ExternalInput").ap()
    din("xs", [5, s_len * 128], BF16)
    din("wbd_hh0", [128, 512], BF16)
    din("wbd_hh1", [128, 512], BF16)
    din("wbd_ih1", [128, 512], BF16)
    din("rc0", [5, 512], BF16)
    din("rc1", [1, 512], BF16)
    din("id128", [128, 128], BF16)
    din("fc1T", [32, 16], BF16)
    din("fc1b", [16, 1], F32)
    din("fc2T", [16, 1], BF16)
    din("fc2b", [1, 1], F32)
    y = nc.dram_tensor("y", [BC, 1], F32, kind="ExternalOutput").ap()

    with tile.TileContext(nc) as tc:
        with ExitStack() as ctx:
            singles = ctx.enter_context(tc.tile_pool(name="singles", bufs=1))
            psmm = ctx.enter_context(tc.tile_pool(name="psmm", bufs=2, space="PSUM"))
            pstr = ctx.enter_context(tc.tile_pool(name="pstr", bufs=2, space="PSUM"))
            psml = ctx.enter_context(tc.tile_pool(name="psml", bufs=1, space="PSUM"))
            sigp = ctx.enter_context(tc.tile_pool(name="sigp", bufs=2))
            smallp = ctx.enter_context(tc.tile_pool(name="smallp", bufs=3))
            xsp = ctx.enter_context(tc.tile_pool(name="xsp", bufs=8))

            # ---- load constants ----
            def load(name, shape, dt):
                t = singles.tile(shape, dt, tag=name)
                nc.sync.dma_start(t[:], d[name][:, :])
                return t
            wbd_hh0 = load("wbd_hh0", [128, 512], BF16)
            wbd_hh1 = load("wbd_hh1", [128, 512], BF16)
            wbd_ih1 = load("wbd_ih1", [128, 512], BF16)
            rc0 = load("rc0", [5, 512], BF16)
            rc1 = load("rc1", [1, 512], BF16)
            id128 = load("id128", [128, 128], BF16)
            fc1T = load("fc1T", [32, 16], BF16)
            fc1b = load("fc1b", [16, 1], F32)
            fc2T = load("fc2T", [16, 1], BF16)
            fc2b = load("fc2b", [1, 1], F32)

            ones1 = singles.tile([1, 128], BF16)
            nc.vector.memset(ones1[:], 1.0)

            # persistent state
            hT0 = singles.tile([128, 128], BF16)
            hT1 = singles.tile([128, 128], BF16)
            c_all = singles.tile([128, 2, NG, H], F32)
            nc.vector.memset(hT0[:], 0.0)
            nc.vector.memset(hT1[:], 0.0)
            nc.vector.memset(c_all[:], 0.0)

            for n in range(s_len + 1):
                do0 = n < s_len
                do1 = n >= 1
                ps = psmm.tile([128, 2, NG, 128], F32)
                if do0:
                    xs_t = xsp.tile([5, 128], BF16)
                    nc.sync.dma_start(xs_t[:], d["xs"][:, n * 128:(n + 1) * 128])
                    nc.tensor.matmul(ps[:, 0], xs_t[:], rc0[:], start=True, stop=False)
                    nc.tensor.matmul(ps[:, 0], hT0[:], wbd_hh0[:], start=False, stop=True)
                if do1:
                    nc.tensor.matmul(ps[:, 1], ones1[:], rc1[:], start=True, stop=False)
                    nc.tensor.matmul(ps[:, 1], hT1[:], wbd_hh1[:], start=False, stop=False)
                    nc.tensor.matmul(ps[:, 1], hT0[:], wbd_ih1[:], start=False, stop=True)

                lsl = slice(0 if do0 else 1, 2 if do1 else 1)  # active layers
                sig = sigp.tile([128, 2, NG, 128], BF16)
                nc.scalar.activation(sig[:, lsl], ps[:, lsl], AF.Sigmoid)
                # g-tilde = 2*sig - 1 (tanh of pre-scaled g-gate)
                nc.vector.tensor_scalar(sig[:, lsl, :, 96:128], sig[:, lsl, :, 96:128],
                                        2.0, -1.0, OP.mult, OP.add)
                u = smallp.tile([128, 2, NG, H], BF16, tag="u")
                nc.vector.tensor_tensor(u[:, lsl], sig[:, lsl, :, 0:32],
                                        sig[:, lsl, :, 96:128], OP.mult)
                nc.vector.tensor_tensor(c_all[:, lsl], sig[:, lsl, :, 32:64],
                                        c_all[:, lsl], OP.mult)
                nc.vector.tensor_tensor(c_all[:, lsl], c_all[:, lsl], u[:, lsl], OP.add)
                th = smallp.tile([128, 2, NG, H], BF16, tag="th")
                nc.scalar.activation(th[:, lsl], c_all[:, lsl], AF.Tanh)
                h = smallp.tile([128, 2, NG, H], BF16, tag="h")
                nc.vector.tensor_tensor(h[:, lsl], sig[:, lsl, :, 64:96],
                                        th[:, lsl], OP.mult)
                if do0:
                    pt0 = pstr.tile([128, 128], BF16, tag="pt")
                    nc.tensor.transpose(pt0[:], h[:, 0], id128[:])
                    nc.vector.tensor_copy(hT0[:], pt0[:])
                if do1:
                    pt1 = pstr.tile([128, 128], BF16, tag="pt")
                    nc.tensor.transpose(pt1[:], h[:, 1], id128[:])
                    nc.vector.tensor_copy(hT1[:], pt1[:])

            # ---- MLP head on h1_last (= hT1) ----
            hstack = smallp.tile([32, 512], BF16, tag="hstack")
            for g in range(NG):
                nc.sync.dma_start(hstack[0:32, g * 128:(g + 1) * 128],
                                  hT1[32 * g:32 * (g + 1), :])
            pm1 = psml.tile([16, 512], F32, tag="mlp1")
            nc.tensor.matmul(pm1[:], fc1T[:], hstack[:], start=True, stop=True)
            z1 = smallp.tile([16, 512], F32, tag="z1")
            nc.scalar.activation(z1[:], pm1[:], AF.Identity, bias=fc1b[:])
            a1 = smallp.tile([16, 512], BF16, tag="a1")
            nc.vector.tensor_scalar(a1[:], z1[:], 0.2, None, OP.mult)
            nc.vector.tensor_tensor(a1[:], z1[:], a1[:], OP.max)
            pm2 = psml.tile([1, 512], F32, tag="mlp2")
            nc.tensor.matmul(pm2[:], fc2T[:], a1[:], start=True, stop=True)
            ysb = smallp.tile([1, 512], F32, tag="ysb")
            nc.scalar.activation(ysb[:], pm2[:], AF.Identity, bias=fc2b[:])
            nc.sync.dma_start(y[:, :], ysb[:])

    nc.compile()
    return nc


_CACHE = {}


def _get_program():
    if "nc" not in _CACHE:
        _CACHE["nc"] = build_program(S_FULL)
    return _CACHE["nc"]


def kernel(x, w_ih0, w_hh0, b_ih0, b_hh0, w_ih1, w_hh1, b_ih1, b_hh1,
           fc1_w, fc1_b, fc2_w, fc2_b):
    x = np.asarray(x, np.float32)
    shared = prep_shared_weights(
        np.asarray(w_ih0, np.float32), np.asarray(w_hh0, np.float32),
        np.asarray(b_ih0, np.float32), np.asarray(b_hh0, np.float32),
        np.asarray(w_ih1, np.float32), np.asarray(w_hh1, np.float32),
        np.asarray(b_ih1, np.float32), np.asarray(b_hh1, np.float32),
        np.asarray(fc1_w, np.float32), np.asarray(fc1_b, np.float32),
        np.asarray(fc2_w, np.float32), np.asarray(fc2_b, np.float32))
    nc = _get_program()
    in_maps = []
    for c in range(NCORES):
        xc = x[c * BC:(c + 1) * BC, :, 0]          # [512, 512]
        m = dict(shared)
        m["xs"] = prep_core_x(xc, S_FULL)
        in_maps.append(m)
    res = bass_utils.run_bass_kernel_spmd(
        nc, in_maps, core_ids=list(range(NCORES)),
        trace=bool(int(os.environ.get("KERNEL_TRACE", "0"))))
    _CACHE["last_results"] = res
    y = np.concatenate([res.results[c]["y"] for c in range(NCORES)], axis=0)
    return y.astype(np.float32)


# revision 22
# speedup vs baseline: 1.0322x; 1.0322x over previous
"""Trainium2 Bass kernel for 2-layer LSTM (H=32, in=1) + MLP head.

Problem: x [4096, 512, 1] f32 -> y [4096, 1] f32.
Strategy: pure data parallel over 8 cores (512 batch each).

Per-core layout ("orientation B", block-diagonal):
  batch b = g*128 + j  (g in 0..3 groups, j in 0..127)
  partition dim = j always; free dim = (layer, group, gate/hidden)

Per tick n (0..512): layer-0 step n and layer-1 step n-1.
  Gates via matmuls with stationary hT (transposed hidden state) and
  streamed block-diagonal weights [128, 512]; x/bias terms via small-K
  matmuls accumulating into the same PSUM banks.
  Gate order (after host-side reorder): [i, f, o, g] per 128-block; the
  g-gate's weights/biases are pre-scaled by 2 so that one big Sigmoid
  covers all gates (tanh(z) = 2*sigmoid(2z) - 1, fixed up on DVE).
  h -> hT via PE transpose (bf16) + DVE copy back to SBUF.
"""

import os
import numpy as np
import ml_dtypes
from contextlib import ExitStack

import concourse.bass as bass
import concourse.tile as tile
import concourse.bacc as bacc
from concourse import mybir
from concourse import bass_utils

F32 = mybir.dt.float32
BF16 = mybir.dt.bfloat16
AF = mybir.ActivationFunctionType
OP = mybir.AluOpType

H = 32
NCORES = 8
B_FULL = 4096
S_FULL = 512
BC = 512          # batch per core
NG = 4            # groups of 128 within the core batch

# torch gate order i,f,g,o -> ours [i,f,o,g]
GATE_PERM = np.r_[0:32, 32:64, 96:128, 64:96]


def _bf(x):
    return np.asarray(x, dtype=ml_dtypes.bfloat16)


def prep_shared_weights(w_ih0, w_hh0, b_ih0, b_hh0, w_ih1, w_hh1, b_ih1, b_hh1,
                        fc1_w, fc1_b, fc2_w, fc2_b):
    """Host-side preparation of the shared (replicated) weight tensors."""
    def reord(w):  # [128, k] -> permuted rows
        return w[GATE_PERM].astype(np.float32)

    w_hh0p = reord(w_hh0)            # [128, 32]
    w_ih0p = reord(w_ih0)            # [128, 1]
    b0p = (b_ih0 + b_hh0)[GATE_PERM].astype(np.float32)  # [128]
    w_hh1p = reord(w_hh1)
    w_ih1p = reord(w_ih1)
    b1p = (b_ih1 + b_hh1)[GATE_PERM].astype(np.float32)

    def blockdiag(wp):  # wp [128 gates, K] -> [NG*K, NG*128]
        k = wp.shape[1]
        out = np.zeros((NG * k, NG * 128), np.float32)
        for g in range(NG):
            out[g * k:(g + 1) * k, g * 128:(g + 1) * 128] = wp.T
        return out

    wbd_hh0 = blockdiag(w_hh0p)      # [128, 512]
    wbd_hh1 = blockdiag(w_hh1p)      # [128, 512]
    wbd_ih1 = blockdiag(w_ih1p)      # [128, 512]
    # const rhs for layer0: rows 0..3 = x block-diag w_ih0 row, row 4 = bias0
    rc0 = np.zeros((5, NG * 128), np.float32)
    for g in range(NG):
        rc0[g, g * 128:(g + 1) * 128] = w_ih0p[:, 0]
    rc0[4] = np.tile(b0p, NG)
    rc1 = np.tile(b1p, NG)[None, :]  # [1, 512]

    return {
        "wbd_hh0": _bf(wbd_hh0),
        "wbd_hh1": _bf(wbd_hh1),
        "wbd_ih1": _bf(wbd_ih1),
        "rc0": _bf(rc0),
        "rc1": _bf(rc1),
        "id128": _bf(np.eye(128, dtype=np.float32)),
        "fc1T": _bf(fc1_w.T.copy()),               # [32, 16]
        "fc1b": fc1_b.reshape(16, 1).astype(np.float32),
        "fc2T": _bf(fc2_w.T.copy()),               # [16, 1]
        "fc2b": fc2_b.reshape(1, 1).astype(np.float32),
    }


def prep_core_x(x_core, s_len):
    """x_core [512, s_len] f32 -> xs [5, s_len*128] bf16 (rows 0..3 x per
    group time-major, row 4 ones)."""
    xs = np.ones((5, s_len * 128), np.float32)
    xr = x_core.reshape(NG, 128, s_len)             # [g, j, t]
    xs[:4] = xr.transpose(0, 2, 1).reshape(NG, s_len * 128)  # [g, t*128+j]
    return _bf(xs)


def build_program(s_len, num_devices=NCORES):
    nc = bacc.Bacc("TRN2", target_bir_lowering=False, debug=False,
                   enable_asserts=False, num_devices=num_devices)
    d = {}
    def din(name, shape, dt):
        d[name] = nc.dram_tensor(name, shape, dt, kind="ExternalInput").ap()
    din("xs", [5, s_len * 128], BF16)
    din("wbd_hh0", [128, 512], BF16)
    din("wbd_hh1", [128, 512], BF16)
    din("wbd_ih1", [128, 512], BF16)
    din("rc0", [5, 512], BF16)
    din("rc1", [1, 512], BF16)
    din("id128", [128, 128], BF16)
    din("fc1T", [32, 16], BF16)
    din("fc1b", [16, 1], F32)
    din("fc2T", [16, 1], BF16)
    din("fc2b", [1, 1], F32)
    y = nc.dram_tensor("y", [BC, 1], F32, kind="ExternalOutput").ap()

    with tile.TileContext(nc) as tc:
        with ExitStack() as ctx:
            singles = ctx.enter_context(tc.tile_pool(name="singles", bufs=1))
            psmm = ctx.enter_context(tc.tile_pool(name="psmm", bufs=2, space="PSUM"))
            pstr = ctx.enter_context(tc.tile_pool(name="pstr", bufs=1, space="PSUM"))
            psml = ctx.enter_context(tc.tile_pool(name="psml", bufs=1, space="PSUM"))
            sigp = ctx.enter_context(tc.tile_pool(name="sigp", bufs=2))
            smallp = ctx.enter_context(tc.tile_pool(name="smallp", bufs=3))
            xsp = ctx.enter_context(tc.tile_pool(name="xsp", bufs=8))

            # ---- load constants ----
            def load(name, shape, dt):
                t = singles.tile(shape, dt, tag=name)
                nc.sync.dma_start(t[:], d[name][:, :])
                return t
            wbd_hh0 = load("wbd_hh0", [128, 512], BF16)
            wbd_hh1 = load("wbd_hh1", [128, 512], BF16)
            wbd_ih1 = load("wbd_ih1", [128, 512], BF16)
            rc0 = load("rc0", [5, 512], BF16)
            rc1 = load("rc1", [1, 512], BF16)
            id128 = load("id128", [128, 128], BF16)
            fc1T = load("fc1T", [32, 16], BF16)
            fc1b = load("fc1b", [16, 1], F32)
            fc2T = load("fc2T", [16, 1], BF16)
            fc2b = load("fc2b", [1, 1], F32)

            ones1 = singles.tile([1, 128], BF16)
            nc.vector.memset(ones1[:], 1.0)

            # persistent state
            hT0 = singles.tile([128, 128], BF16)
            hT1 = singles.tile([128, 128], BF16)
            c0 = singles.tile([128, NG, H], F32)
            c1 = singles.tile([128, NG, H], F32)
            nc.vector.memset(hT0[:], 0.0)
            nc.vector.memset(hT1[:], 0.0)
            nc.vector.memset(c0[:], 0.0)
            nc.vector.memset(c1[:], 0.0)

            for n in range(s_len + 1):
                do0 = n < s_len
                do1 = n >= 1
                ps = psmm.tile([128, 2, NG, 128], F32)
                if do0:
                    xs_t = xsp.tile([5, 128], BF16)
                    nc.sync.dma_start(xs_t[:], d["xs"][:, n * 128:(n + 1) * 128])
                    nc.tensor.matmul(ps[:, 0], xs_t[:], rc0[:], start=True, stop=False)
                    nc.tensor.matmul(ps[:, 0], hT0[:], wbd_hh0[:], start=False, stop=True)
                if do1:
                    nc.tensor.matmul(ps[:, 1], ones1[:], rc1[:], start=True, stop=False)
                    nc.tensor.matmul(ps[:, 1], hT1[:], wbd_hh1[:], start=False, stop=False)
                    nc.tensor.matmul(ps[:, 1], hT0[:], wbd_ih1[:], start=False, stop=True)

                lsl = slice(0 if do0 else 1, 2 if do1 else 1)  # active layers
                sig = sigp.tile([128, 2, NG, 128], BF16)
                nc.scalar.activation(sig[:, lsl, :, 0:96], ps[:, lsl, :, 0:96],
                                     AF.Sigmoid)
                nc.scalar.activation(sig[:, lsl, :, 96:128], ps[:, lsl, :, 96:128],
                                     AF.Tanh)
                # per-layer independent chains from here on
                for l, (dol, c_l, hT_l) in enumerate(
                        [(do0, c0, hT0), (do1, c1, hT1)]):
                    if not dol:
                        continue
                    u = smallp.tile([128, NG, H], BF16, tag=f"u{l}")
                    nc.gpsimd.tensor_tensor(u[:], sig[:, l, :, 0:32],
                                            sig[:, l, :, 96:128], OP.mult)
                    nc.vector.tensor_tensor(c_l[:], sig[:, l, :, 32:64],
                                            c_l[:], OP.mult)
                    nc.vector.tensor_tensor(c_l[:], c_l[:], u[:], OP.add)
                    th = smallp.tile([128, NG, H], BF16, tag=f"th{l}")
                    nc.scalar.activation(th[:], c_l[:], AF.Tanh)
                    h = smallp.tile([128, NG, H], BF16, tag=f"h{l}")
                    nc.vector.tensor_tensor(h[:], sig[:, l, :, 64:96],
                                            th[:], OP.mult)
                    pt = pstr.tile([128, 128], BF16, tag=f"pt{l}")
                    nc.tensor.transpose(pt[:], h[:], id128[:])
                    nc.vector.tensor_copy(hT_l[:], pt[:])

            # ---- MLP head on h1_last (= hT1) ----
            hstack = smallp.tile([32, 512], BF16, tag="hstack")
            for g in range(NG):
                nc.sync.dma_start(hstack[0:32, g * 128:(g + 1) * 128],
                                  hT1[32 * g:32 * (g + 1), :])
            pm1 = psml.tile([16, 512], F32, tag="mlp1")
            nc.tensor.matmul(pm1[:], fc1T[:], hstack[:], start=True, stop=True)
            z1 = smallp.tile([16, 512], F32, tag="z1")
            nc.scalar.activation(z1[:], pm1[:], AF.Identity, bias=fc1b[:])
            a1 = smallp.tile([16, 512], BF16, tag="a1")
            nc.vector.tensor_scalar(a1[:], z1[:], 0.2, None, OP.mult)
            nc.vector.tensor_tensor(a1[:], z1[:], a1[:], OP.max)
            pm2 = psml.tile([1, 512], F32, tag="mlp2")
            nc.tensor.matmul(pm2[:], fc2T[:], a1[:], start=True, stop=True)
            ysb = smallp.tile([1, 512], F32, tag="ysb")
            nc.scalar.activation(ysb[:], pm2[:], AF.Identity, bias=fc2b[:])
            nc.sync.dma_start(y[:, :], ysb[:])

    nc.compile()
    return nc


_CACHE = {}


def _get_program():
    if "nc" not in _CACHE:
        _CACHE["nc"] = build_program(S_FULL)
    return _CACHE["nc"]


def kernel(x, w_ih0, w_hh0, b_ih0, b_hh0, w_ih1, w_hh1, b_ih1, b_hh1,
           fc1_w, fc1_b, fc2_w, fc2_b):
    x = np.asarray(x, np.float32)
    shared = prep_shared_weights(
        np.asarray(w_ih0, np.float32), np.asarray(w_hh0, np.float32),
        np.asarray(b_ih0, np.float32), np.asarray(b_hh0, np.float32),
        np.asarray(w_ih1, np.float32), np.asarray(w_hh1, np.float32),
        np.asarray(b_ih1, np.float32), np.asarray(b_hh1, np.float32),
        np.asarray(fc1_w, np.float32), np.asarray(fc1_b, np.float32),
        np.asarray(fc2_w, np.float32), np.asarray(fc2_b, np.float32))
    nc = _get_program()
    in_maps = []
    for c in range(NCORES):
        xc = x[c * BC:(c + 1) * BC, :, 0]          # [512, 512]
        m = dict(shared)
        m["xs"] = prep_core_x(xc, S_FULL)
        in_maps.append(m)
    res = bass_utils.run_bass_kernel_spmd(
        nc, in_maps, core_ids=list(range(NCORES)),
        trace=bool(int(os.environ.get("KERNEL_TRACE", "0"))))
    _CACHE["last_results"] = res
    y = np.concatenate([res.results[c]["y"] for c in range(NCORES)], axis=0)
    return y.astype(np.float32)


# revision 24
# speedup vs baseline: 1.3039x; 1.2631x over previous
"""Trainium2 Bass kernel for 2-layer LSTM (H=32, in=1) + MLP head.

Problem: x [4096, 512, 1] f32 -> y [4096, 1] f32.
Strategy: pure data parallel over 8 cores (512 batch each).

Per-core layout ("orientation B", block-diagonal):
  batch b = g*128 + j  (g in 0..3 groups, j in 0..127)
  partition dim = j always; free dim = (layer, group, gate/hidden)

Per tick n (0..512): layer-0 step n and layer-1 step n-1.
  Gates via matmuls with stationary hT (transposed hidden state) and
  streamed block-diagonal weights [128, 512]; x/bias terms via small-K
  matmuls accumulating into the same PSUM banks.
  Gate order (after host-side reorder): [i, f, o, g] per 128-block; the
  g-gate's weights/biases are pre-scaled by 2 so that one big Sigmoid
  covers all gates (tanh(z) = 2*sigmoid(2z) - 1, fixed up on DVE).
  h -> hT via PE transpose (bf16) + DVE copy back to SBUF.
"""

import os
import numpy as np
import ml_dtypes
from contextlib import ExitStack

import concourse.bass as bass
import concourse.tile as tile
import concourse.bacc as bacc
from concourse import mybir
from concourse import bass_utils

F32 = mybir.dt.float32
BF16 = mybir.dt.bfloat16
AF = mybir.ActivationFunctionType
OP = mybir.AluOpType

H = 32
NCORES = 8
B_FULL = 4096
S_FULL = 512
BC = 512          # batch per core
NG = 4            # groups of 128 within the core batch

# torch gate order i,f,g,o -> ours [i,f,o,g]
GATE_PERM = np.r_[0:32, 32:64, 96:128, 64:96]


def _bf(x):
    return np.asarray(x, dtype=ml_dtypes.bfloat16)


def prep_shared_weights(w_ih0, w_hh0, b_ih0, b_hh0, w_ih1, w_hh1, b_ih1, b_hh1,
                        fc1_w, fc1_b, fc2_w, fc2_b):
    """Host-side preparation of the shared (replicated) weight tensors."""
    def reord(w):  # [128, k] -> permuted rows
        return w[GATE_PERM].astype(np.float32)

    w_hh0p = reord(w_hh0)            # [128, 32]
    w_ih0p = reord(w_ih0)            # [128, 1]
    b0p = (b_ih0 + b_hh0)[GATE_PERM].astype(np.float32)  # [128]
    w_hh1p = reord(w_hh1)
    w_ih1p = reord(w_ih1)
    b1p = (b_ih1 + b_hh1)[GATE_PERM].astype(np.float32)

    def blockdiag(wp):  # wp [128 gates, K] -> [NG*K, NG*128]
        k = wp.shape[1]
        out = np.zeros((NG * k, NG * 128), np.float32)
        for g in range(NG):
            out[g * k:(g + 1) * k, g * 128:(g + 1) * 128] = wp.T
        return out

    wbd_hh0 = blockdiag(w_hh0p)      # [128, 512]
    wbd_hh1 = blockdiag(w_hh1p)      # [128, 512]
    wbd_ih1 = blockdiag(w_ih1p)      # [128, 512]
    # const rhs for layer0: rows 0..3 = x block-diag w_ih0 row, row 4 = bias0
    rc0 = np.zeros((5, NG * 128), np.float32)
    for g in range(NG):
        rc0[g, g * 128:(g + 1) * 128] = w_ih0p[:, 0]
    rc0[4] = np.tile(b0p, NG)
    rc1 = np.tile(b1p, NG)[None, :]  # [1, 512]

    return {
        "wbd_hh0": _bf(wbd_hh0),
        "wbd_hh1": _bf(wbd_hh1),
        "wbd_ih1": _bf(wbd_ih1),
        "rc0": _bf(rc0),
        "rc1": _bf(rc1),
        "id128": _bf(np.eye(128, dtype=np.float32)),
        "fc1T": _bf(fc1_w.T.copy()),               # [32, 16]
        "fc1b": fc1_b.reshape(16, 1).astype(np.float32),
        "fc2T": _bf(fc2_w.T.copy()),               # [16, 1]
        "fc2b": fc2_b.reshape(1, 1).astype(np.float32),
    }


def prep_core_x(x_core, s_len):
    """x_core [512, s_len] f32 -> xs [5, s_len*128] bf16 (rows 0..3 x per
    group time-major, row 4 ones)."""
    xs = np.ones((5, s_len * 128), np.float32)
    xr = x_core.reshape(NG, 128, s_len)             # [g, j, t]
    xs[:4] = xr.transpose(0, 2, 1).reshape(NG, s_len * 128)  # [g, t*128+j]
    return _bf(xs)


def build_program(s_len, num_devices=NCORES):
    nc = bacc.Bacc("TRN2", target_bir_lowering=False, debug=False,
                   enable_asserts=False, num_devices=num_devices)
    d = {}
    def din(name, shape, dt):
        d[name] = nc.dram_tensor(name, shape, dt, kind="ExternalInput").ap()
    din("xs", [5, s_len * 128], BF16)
    din("wbd_hh0", [128, 512], BF16)
    din("wbd_hh1", [128, 512], BF16)
    din("wbd_ih1", [128, 512], BF16)
    din("rc0", [5, 512], BF16)
    din("rc1", [1, 512], BF16)
    din("id128", [128, 128], BF16)
    din("fc1T", [32, 16], BF16)
    din("fc1b", [16, 1], F32)
    din("fc2T", [16, 1], BF16)
    din("fc2b", [1, 1], F32)
    y = nc.dram_tensor("y", [BC, 1], F32, kind="ExternalOutput").ap()

    with tile.TileContext(nc) as tc:
        with ExitStack() as ctx:
            singles = ctx.enter_context(tc.tile_pool(name="singles", bufs=1))
            psmm = ctx.enter_context(tc.tile_pool(name="psmm", bufs=2, space="PSUM"))
            pstr = ctx.enter_context(tc.tile_pool(name="pstr", bufs=1, space="PSUM"))
            psml = ctx.enter_context(tc.tile_pool(name="psml", bufs=1, space="PSUM"))
            sigp = ctx.enter_context(tc.tile_pool(name="sigp", bufs=2))
            smallp = ctx.enter_context(tc.tile_pool(name="smallp", bufs=3))
            xsp = ctx.enter_context(tc.tile_pool(name="xsp", bufs=8))

            # ---- load constants ----
            def load(name, shape, dt):
                t = singles.tile(shape, dt, tag=name)
                nc.sync.dma_start(t[:], d[name][:, :])
                return t
            wbd_hh0 = load("wbd_hh0", [128, 512], BF16)
            wbd_hh1 = load("wbd_hh1", [128, 512], BF16)
            wbd_ih1 = load("wbd_ih1", [128, 512], BF16)
            rc0 = load("rc0", [5, 512], BF16)
            rc1 = load("rc1", [1, 512], BF16)
            id128 = load("id128", [128, 128], BF16)
            fc1T = load("fc1T", [32, 16], BF16)
            fc1b = load("fc1b", [16, 1], F32)
            fc2T = load("fc2T", [16, 1], BF16)
            fc2b = load("fc2b", [1, 1], F32)

            ones1 = singles.tile([1, 128], BF16)
            nc.vector.memset(ones1[:], 1.0)

            # persistent state
            hT0 = singles.tile([128, 128], BF16)
            hT1 = singles.tile([128, 128], BF16)
            c0 = singles.tile([128, NG, H], F32)
            c1 = singles.tile([128, NG, H], F32)
            nc.vector.memset(hT0[:], 0.0)
            nc.vector.memset(hT1[:], 0.0)
            nc.vector.memset(c0[:], 0.0)
            nc.vector.memset(c1[:], 0.0)

            for n in range(s_len + 1):
                do0 = n < s_len
                do1 = n >= 1
                ps0 = ps1 = None
                if do0:
                    ps0 = psmm.tile([128, NG, 128], F32, tag="ps0")
                if do1:
                    ps1 = psmm.tile([128, NG, 128], F32, tag="ps1")
                if do0:
                    xs_t = xsp.tile([5, 128], BF16)
                    nc.sync.dma_start(xs_t[:], d["xs"][:, n * 128:(n + 1) * 128])
                    nc.tensor.matmul(ps0[:], xs_t[:], rc0[:], start=True, stop=False)
                    nc.tensor.matmul(ps0[:], hT0[:], wbd_hh0[:], start=False, stop=True)
                if do1:
                    nc.tensor.matmul(ps1[:], ones1[:], rc1[:], start=True, stop=False)
                    nc.tensor.matmul(ps1[:], hT1[:], wbd_hh1[:], start=False, stop=False)
                    nc.tensor.matmul(ps1[:], hT0[:], wbd_ih1[:], start=False, stop=True)

                # fully independent per-layer chains
                for l, (dol, ps_l, c_l, hT_l) in enumerate(
                        [(do0, ps0, c0, hT0), (do1, ps1, c1, hT1)]):
                    if not dol:
                        continue
                    sig = sigp.tile([128, NG, 128], BF16, tag=f"sig{l}")
                    nc.scalar.activation(sig[:, :, 0:96], ps_l[:, :, 0:96],
                                         AF.Sigmoid)
                    nc.scalar.activation(sig[:, :, 96:128], ps_l[:, :, 96:128],
                                         AF.Tanh)
                    u = smallp.tile([128, NG, H], BF16, tag=f"u{l}")
                    nc.vector.tensor_tensor(u[:], sig[:, :, 0:32],
                                            sig[:, :, 96:128], OP.mult)
                    nc.vector.tensor_tensor(c_l[:], sig[:, :, 32:64],
                                            c_l[:], OP.mult)
                    nc.vector.tensor_tensor(c_l[:], c_l[:], u[:], OP.add)
                    th = smallp.tile([128, NG, H], BF16, tag=f"th{l}")
                    nc.scalar.activation(th[:], c_l[:], AF.Tanh)
                    h = smallp.tile([128, NG, H], BF16, tag=f"h{l}")
                    nc.vector.tensor_tensor(h[:], sig[:, :, 64:96],
                                            th[:], OP.mult)
                    pt = pstr.tile([128, 128], BF16, tag=f"pt{l}")
                    nc.tensor.transpose(pt[:], h[:], id128[:])
                    nc.vector.tensor_copy(hT_l[:], pt[:])

            # ---- MLP head on h1_last (= hT1) ----
            hstack = smallp.tile([32, 512], BF16, tag="hstack")
            for g in range(NG):
                nc.sync.dma_start(hstack[0:32, g * 128:(g + 1) * 128],
                                  hT1[32 * g:32 * (g + 1), :])
            pm1 = psml.tile([16, 512], F32, tag="mlp1")
            nc.tensor.matmul(pm1[:], fc1T[:], hstack[:], start=True, stop=True)
            z1 = smallp.tile([16, 512], F32, tag="z1")
            nc.scalar.activation(z1[:], pm1[:], AF.Identity, bias=fc1b[:])
            a1 = smallp.tile([16, 512], BF16, tag="a1")
            nc.vector.tensor_scalar(a1[:], z1[:], 0.2, None, OP.mult)
            nc.vector.tensor_tensor(a1[:], z1[:], a1[:], OP.max)
            pm2 = psml.tile([1, 512], F32, tag="mlp2")
            nc.tensor.matmul(pm2[:], fc2T[:], a1[:], start=True, stop=True)
            ysb = smallp.tile([1, 512], F32, tag="ysb")
            nc.scalar.activation(ysb[:], pm2[:], AF.Identity, bias=fc2b[:])
            nc.sync.dma_start(y[:, :], ysb[:])

    nc.compile()
    return nc


_CACHE = {}


def _get_program():
    if "nc" not in _CACHE:
        _CACHE["nc"] = build_program(S_FULL)
    return _CACHE["nc"]


def kernel(x, w_ih0, w_hh0, b_ih0, b_hh0, w_ih1, w_hh1, b_ih1, b_hh1,
           fc1_w, fc1_b, fc2_w, fc2_b):
    x = np.asarray(x, np.float32)
    shared = prep_shared_weights(
        np.asarray(w_ih0, np.float32), np.asarray(w_hh0, np.float32),
        np.asarray(b_ih0, np.float32), np.asarray(b_hh0, np.float32),
        np.asarray(w_ih1, np.float32), np.asarray(w_hh1, np.float32),
        np.asarray(b_ih1, np.float32), np.asarray(b_hh1, np.float32),
        np.asarray(fc1_w, np.float32), np.asarray(fc1_b, np.float32),
        np.asarray(fc2_w, np.float32), np.asarray(fc2_b, np.float32))
    nc = _get_program()
    in_maps = []
    for c in range(NCORES):
        xc = x[c * BC:(c + 1) * BC, :, 0]          # [512, 512]
        m = dict(shared)
        m["xs"] = prep_core_x(xc, S_FULL)
        in_maps.append(m)
    res = bass_utils.run_bass_kernel_spmd(
        nc, in_maps, core_ids=list(range(NCORES)),
        trace=bool(int(os.environ.get("KERNEL_TRACE", "0"))))
    _CACHE["last_results"] = res
    y = np.concatenate([res.results[c]["y"] for c in range(NCORES)], axis=0)
    return y.astype(np.float32)
